# revision 32
# baseline (speedup 1.0000x reference)
"""Trainium2 Bass kernel for nn_CLM_26594437496868 (co-attention + conv/BN/leakyrelu).

Reference computation (b=4, c=64, h=w=64, hw=4096):
  EL = W_lin @ E                       # [c, hw] per sample
  A[n, m] = sum_c EL[c, n] Q[c, m]     # [hw, hw]
  query_c[c, n]    = sum_m Q[c, m] exp(A[n, m]) / sum_m exp(A[n, m])
  exemplar_c[c, n] = sum_m E[c, m] exp(A[m, n]) / sum_m exp(A[m, n])
  out_x = query_c + exemplar_c + E + Q
  y = conv3x3(out_x, W_conv); y = BN(y) * gamma + beta; leaky_relu(y, 0.1)

Sharding: 8 cores = 4 samples x 2 image-halves (rows 0-31 / 32-63).
Each core computes BOTH attention orientations for its 34-row slice
(rows R0-1 .. R0+32, one halo row each side, phantom rows zero-padded
by the host and masked out on device), the conv for all 64 output
channels of its 32 output rows, and local BN partial stats.  One tiny
AllReduce ([64,2] fp32) combines BN stats across all 8 cores.
"""
import sys
if "/opt/trn_rl_repo" not in sys.path:
    sys.path.append("/opt/trn_rl_repo")

import numpy as np

import concourse.bass as bass
import concourse.bacc as bacc
import concourse.tile as tile
import concourse.masks as masks
from concourse import mybir
from concourse import bass_utils

N_CORES = 8
C = 64                    # channels
HW = 4096                 # 64*64
W_IMG = 64
NH = 2176                 # 34 rows * 64 cols  (1 halo row each side)
NOUT = 2048               # 32 output rows * 64
N_BLOCKS = [(0, 512), (512, 512), (1024, 512), (1536, 512), (2048, 128)]
M_CHUNKS = 32             # 4096 / 128
BN_EPS = 1e-5
LEAKY = 0.1

BF16 = mybir.dt.bfloat16
F32 = mybir.dt.float32
NPBF16 = mybir.dt.np(BF16)

_COMPILED = None


def _build_program():
    nc = bacc.Bacc("TRN2", target_bir_lowering=False, debug=False,
                   enable_asserts=True, num_devices=N_CORES)

    # ---- I/O ----
    # pack (bf16, per-partition layout), attention-critical prefix first:
    #   [eh | xq | wt | qh | eqh | mask | xe | wconv(9*64)]
    PACKW = 4 * NH + 2 * HW + 9 * C + C
    PACK_CRIT = NH + HW + C
    d_pack = nc.dram_tensor("pack", [C, PACKW], BF16, kind="ExternalInput").ap()
    d_xe = nc.dram_tensor("xe", [C, HW], BF16, kind="ExternalInput").ap()
    d_xq = nc.dram_tensor("xq", [C, HW], BF16, kind="ExternalInput").ap()
    d_gb = nc.dram_tensor("gb", [C, 2], F32, kind="ExternalInput").ap()
    d_out = nc.dram_tensor("out", [C, NOUT], F32, kind="ExternalOutput").ap()

    from contextlib import ExitStack
    with tile.TileContext(nc) as tc, ExitStack() as ctx:
        consts = ctx.enter_context(tc.tile_pool(name="consts", bufs=1))
        big = ctx.enter_context(tc.tile_pool(name="big", bufs=1))
        expp = ctx.enter_context(tc.tile_pool(name="expp", bufs=3))
        smalls = ctx.enter_context(tc.tile_pool(name="smalls", bufs=2))
        dram = ctx.enter_context(tc.tile_pool(name="dram", bufs=1, space="DRAM"))
        ps_strip_cm = tc.tile_pool(name="ps_strip", bufs=2, space="PSUM")
        ps_strip = ps_strip_cm.__enter__()
        ps_pv_cm = tc.tile_pool(name="ps_pv", bufs=1, space="PSUM")
        ps_pv = ps_pv_cm.__enter__()

        # ---- load inputs: split packed DMA (critical prefix first) ----
        pack_sb = big.tile([C, PACKW], BF16)
        nc.sync.dma_start(out=pack_sb[:, 0:PACK_CRIT], in_=d_pack[:, 0:PACK_CRIT])
        nc.sync.dma_start(out=pack_sb[:, PACK_CRIT:], in_=d_pack[:, PACK_CRIT:])
        o0 = 0
        eh_sb = pack_sb[:, o0:o0 + NH]; o0 += NH
        xq_sb = pack_sb[:, o0:o0 + HW]; o0 += HW
        wt_sb = pack_sb[:, o0:o0 + C]; o0 += C
        qh_sb = pack_sb[:, o0:o0 + NH]; o0 += NH
        eqh_sb = pack_sb[:, o0:o0 + NH]; o0 += NH
        mask_sb = pack_sb[:, o0:o0 + NH]; o0 += NH
        xe_sb = pack_sb[:, o0:o0 + HW]; o0 += HW
        wconv_sb = pack_sb[:, o0:o0 + 9 * C].rearrange(
            "p (t o) -> p t o", t=9); o0 += 9 * C
        gb_sb = consts.tile([C, 2], F32)
        nc.sync.dma_start(out=gb_sb[:], in_=d_gb[:])
        gamma_sb = gb_sb[:, 0:1]
        beta_sb = gb_sb[:, 1:2]

        alpha_sb = consts.tile([C, 1], F32)
        nc.vector.memset(alpha_sb[:], LEAKY)
        eps_sb = consts.tile([C, 1], F32)
        nc.vector.memset(eps_sb[:], BN_EPS)
        # warm the ACT exp table while the input DMAs run
        warm_sb = consts.tile([C, 1], F32)
        nc.scalar.activation(out=warm_sb[:], in_=eps_sb[:],
                             func=mybir.ActivationFunctionType.Exp)

        # ---- EL-half = W_lin @ eh (gates the first attention strip) ----
        elf_sb = big.tile([C, HW], BF16)        # full-m EL for orientation 2 lhsT
        elh_sb = big.tile([C, NH], BF16)        # half-n EL for orientation 1 rhs
        for (off, nb) in N_BLOCKS:
            ps_el = ps_strip.tile([128, 3, 512], F32, tag="sp")
            nc.tensor.matmul(ps_el[0:C, 0, 0:nb], wt_sb[:],
                             eh_sb[:, off:off + nb], start=True, stop=True)
            nc.vector.tensor_copy(elh_sb[:, off:off + nb], ps_el[0:C, 0, 0:nb])

        # [Q^T | 1] via xbar DMA transpose straight from DRAM
        qtd = big.tile([128, M_CHUNKS, C], BF16)
        etd = big.tile([128, M_CHUNKS, C], BF16)
        nc.sync.dma_start_transpose(out=qtd[:], in_=d_xq[:])
        qt_sb = big.tile([128, M_CHUNKS, C + 1], BF16)
        et_sb = big.tile([128, M_CHUNKS, C + 1], BF16)
        nc.vector.memset(qt_sb[:, :, C:C + 1], 1.0)
        nc.vector.tensor_copy(qt_sb[:, :, 0:C], qtd[:])

        def emit_o1_prep():
            # orientation-1 operands; emitted inside the o=0 window of block 0.
            # Their PSUM lives in the (idle during o=0) pv1 slot.
            for j in range(HW // 512):
                ps_el = ps_pv.tile([C + 1, 512], F32, tag="pv1")
                nc.tensor.matmul(ps_el[0:C, :], wt_sb[:],
                                 xe_sb[:, j * 512:(j + 1) * 512],
                                 start=True, stop=True)
                nc.vector.tensor_copy(elf_sb[:, j * 512:(j + 1) * 512],
                                      ps_el[0:C, :])
            nc.sync.dma_start_transpose(out=etd[:], in_=d_xe[:])
            nc.vector.memset(et_sb[:, :, C:C + 1], 1.0)
            nc.vector.tensor_copy(et_sb[:, :, 0:C], etd[:])

        # ---- conv input (built incrementally): [64, 34 rows, 66 cols] ----
        xpad = big.tile([C, 34, 66], BF16)
        nc.vector.memset(xpad[:], 0.0)

        # ---- attention: both orientations, streamed over m in 3-chunk strips ----
        s_lhs = (xq_sb, elf_sb)       # T1[m,l] = sum_c Q[c,m] ELh[c,l] ; T2 = sum_c EL[c,m] qh[c,l]
        s_rhs = (elh_sb, qh_sb)
        pv_lhs = (qt_sb, et_sb)
        STRIPS = [(0, 3), (3, 3), (6, 3), (9, 3), (12, 3), (15, 3), (18, 3),
                  (21, 3), (24, 3), (27, 3), (30, 2)]  # (chunk0, nchunks)
        y_sb = big.tile([C, NOUT], F32)
        st = smalls.tile([C, 4, 6], F32, tag="st")

        def emit_conv_block(rb, pool):
            # conv out rows 8rb..8rb+7 <- xpad rows 8rb..8rb+9
            yp = pool.tile([C + 1, 512], F32, tag="yp")
            for tap in range(9):
                dy, dx = tap // 3, tap % 3
                nc.tensor.matmul(
                    yp[0:C, :],
                    wconv_sb[:, tap, :],
                    xpad[:, 8 * rb + dy:8 * rb + dy + 8, dx:dx + 64],
                    start=(tap == 0), stop=(tap == 8))
            nc.vector.tensor_copy(y_sb[:, rb * 512:(rb + 1) * 512], yp[0:C, :])
            nc.vector.bn_stats(out=st[:, rb, :],
                               in_=y_sb[:, rb * 512:(rb + 1) * 512])

        for ib, (off, nb) in enumerate(N_BLOCKS):
            nrows = nb // W_IMG
            r0 = off // W_IMG
            pvs = []
            for o in (0, 1):
                pv = ps_pv.tile([C + 1, 512], F32, tag=f"pv{o}")
                pvs.append(pv)
                for (c0, ns) in STRIPS:
                    sp = ps_strip.tile([128, 3, 512], F32, tag="sp")
                    for u in range(ns):
                        j = c0 + u
                        nc.tensor.matmul(sp[:, u, 0:nb],
                                         s_lhs[o][:, 128 * j:128 * j + 128],
                                         s_rhs[o][:, off:off + nb],
                                         start=True, stop=True)
                    ex = expp.tile([128, 3, 512], BF16, tag="ex")
                    nc.scalar.activation(out=ex[:, 0:ns, 0:nb],
                                         in_=sp[:, 0:ns, 0:nb],
                                         func=mybir.ActivationFunctionType.Exp)
                    for u in range(ns):
                        j = c0 + u
                        nc.tensor.matmul(pv[:, 0:nb], pv_lhs[o][:, j, :],
                                         ex[:, u, 0:nb],
                                         start=(j == 0), stop=(j == M_CHUNKS - 1))
                if off == 0 and o == 0:
                    emit_o1_prep()
            # normalize:  O[c, l] * (1/D[l]) ; 1/D broadcast via gpsimd.
            # Copy PSUM->SBUF first so the accumulator bank frees quickly.
            zs = []
            for o in (0, 1):
                pvc = smalls.tile([C + 1, 512], F32, tag=f"pvc{o}")
                nc.vector.tensor_copy(pvc[:, 0:nb], pvs[o][:, 0:nb])
                rd = smalls.tile([128, 512], F32, tag="rd")
                nc.vector.reciprocal(rd[64:65, 0:nb], pvc[C:C + 1, 0:nb])
                rd0 = smalls.tile([1, 512], F32, tag="rd0")
                nc.sync.dma_start(out=rd0[0:1, 0:nb], in_=rd[64:65, 0:nb])
                bc = smalls.tile([C, 512], F32, tag="bc")
                nc.gpsimd.partition_broadcast(bc[:, 0:nb], rd0[0:1, 0:nb])
                z = smalls.tile([C, 512], F32, tag=f"z{o}")
                nc.vector.tensor_mul(z[:, 0:nb], pvc[0:C, 0:nb], bc[:, 0:nb])
                zs.append(z)
            zsum = smalls.tile([C, 512], F32, tag="zsum")
            nc.vector.tensor_add(zsum[:, 0:nb], zs[0][:, 0:nb], zs[1][:, 0:nb])
            nc.vector.tensor_add(zsum[:, 0:nb], zsum[:, 0:nb],
                                 eqh_sb[:, off:off + nb])
            # masked cast into the padded conv input
            nc.vector.scalar_tensor_tensor(
                out=xpad[:, r0:r0 + nrows, 1:65],
                in0=zsum[:, 0:nb].rearrange("p (r w) -> p r w", w=W_IMG),
                scalar=1.0,
                in1=mask_sb[:, off:off + nb].rearrange("p (r w) -> p r w", w=W_IMG),
                op0=mybir.AluOpType.mult,
                op1=mybir.AluOpType.mult,
            )
        # ---- conv 3x3 + BN stats (after attention; PSUM pools swapped) ----
        ps_pv_cm.__exit__(None, None, None)
        ps_strip_cm.__exit__(None, None, None)
        ps_conv = ctx.enter_context(tc.tile_pool(name="ps_conv", bufs=2, space="PSUM"))
        for rb in range(4):
            emit_conv_block(rb, ps_conv)
        mv = smalls.tile([C, 2], F32, tag="mv")
        nc.vector.bn_aggr(out=mv[:], in_=st[:])

        # ---- BN stats AllGather (cheaper than AllReduce) + local reduce ----
        ccs = smalls.tile([C, 2], F32, tag="ccs")
        m2 = smalls.tile([C, 1], F32, tag="m2")
        nc.vector.tensor_mul(m2[:], mv[:, 0:1], mv[:, 0:1])
        nc.vector.tensor_copy(ccs[:, 0:1], mv[:, 0:1])
        nc.vector.tensor_add(ccs[:, 1:2], mv[:, 1:2], m2[:])
        cc_in = dram.tile([C, 2], F32)
        cc_out = dram.tile([N_CORES, C, 2], F32, addr_space="Shared")
        nc.sync.dma_start(out=cc_in[:], in_=ccs[:])
        nc.gpsimd.collective_compute(
            "AllGather", mybir.AluOpType.bypass,
            replica_groups=[list(range(N_CORES))],
            ins=[cc_in.opt()], outs=[cc_out.opt()])
        gath = smalls.tile([C, 2, N_CORES], F32, tag="gath")
        nc.sync.dma_start(out=gath[:],
                          in_=cc_out[:].rearrange("r c v -> c v r"))
        red = smalls.tile([C, 2], F32, tag="red")
        nc.vector.tensor_reduce(red[:], gath[:], axis=mybir.AxisListType.X,
                                op=mybir.AluOpType.add)

        # mu = red0/8 ; var = red1/8 - mu^2 ; rstd = exp(-0.5*ln(var+eps))
        nc.vector.tensor_scalar_mul(red[:], red[:], 1.0 / N_CORES)
        mu = red[:, 0:1]
        var = smalls.tile([C, 1], F32, tag="var")
        mu2 = smalls.tile([C, 1], F32, tag="mu2")
        nc.vector.tensor_mul(mu2[:], mu, mu)
        nc.vector.tensor_sub(var[:], red[:, 1:2], mu2[:])
        lnv = smalls.tile([C, 1], F32, tag="lnv")
        nc.scalar.activation(out=lnv[:], in_=var[:],
                             func=mybir.ActivationFunctionType.Ln, bias=eps_sb[:])
        rstd = smalls.tile([C, 1], F32, tag="rstd")
        nc.scalar.activation(out=rstd[:], in_=lnv[:],
                             func=mybir.ActivationFunctionType.Exp, scale=-0.5)
        scale_f = smalls.tile([C, 1], F32, tag="scale_f")
        bias_f = smalls.tile([C, 1], F32, tag="bias_f")
        nc.vector.tensor_mul(scale_f[:], gamma_sb[:], rstd[:])
        nc.vector.tensor_mul(bias_f[:], mu, scale_f[:])
        nc.vector.tensor_sub(bias_f[:], beta_sb[:], bias_f[:])

        # ---- apply BN + leaky relu, write out ----
        # Prelu respects a per-partition alpha AP (Lrelu ignores its alpha
        # and uses the hardware default 0.01) -> one ACT op per block.
        osb = big.tile([C, NOUT], F32)
        nc.scalar.activation(out=osb[:], in_=y_sb[:],
                             func=mybir.ActivationFunctionType.Prelu,
                             bias=bias_f[:], scale=scale_f[:],
                             alpha=alpha_sb[:])
        nc.sync.dma_start(out=d_out[:], in_=osb[:])

    nc.compile()
    return nc


def _get_program():
    global _COMPILED
    if _COMPILED is None:
        _COMPILED = _build_program()
    return _COMPILED


def _make_in_maps(exemplar, query, W_lin, W_conv, gamma, beta):
    E = np.asarray(exemplar, dtype=np.float32).reshape(4, C, HW)
    Q = np.asarray(query, dtype=np.float32).reshape(4, C, HW)
    wt = np.ascontiguousarray(np.asarray(W_lin, np.float32).T).astype(NPBF16)
    assert wt.shape == (C, C)
    wconv = np.ascontiguousarray(
        np.asarray(W_conv, np.float32).transpose(1, 2, 3, 0).reshape(C, 9, C)
    ).astype(NPBF16)
    g = np.asarray(gamma, np.float32).reshape(C, 1)
    b = np.asarray(beta, np.float32).reshape(C, 1)

    zeros = np.zeros((C, W_IMG), np.float32)
    in_maps = []
    for k in range(N_CORES):
        s, h = divmod(k, 2)
        if h == 0:
            sl = lambda X: np.concatenate([zeros, X[s][:, :NH - W_IMG]], axis=1)
        else:
            sl = lambda X: np.concatenate([X[s][:, HW - (NH - W_IMG):], zeros], axis=1)
        eh = sl(E)
        qh = sl(Q)
        mask = np.ones((C, NH), np.float32)
        if h == 0:
            mask[:, :W_IMG] = 0.0
        else:
            mask[:, NH - W_IMG:] = 0.0
        xe_bf = E[s].astype(NPBF16)
        xq_bf = Q[s].astype(NPBF16)
        # order must match the device-side unpack:
        #   [eh | xq | wt | qh | eqh | mask | xe | wconv]
        pack = np.concatenate([
            eh.astype(NPBF16), xq_bf, wt, qh.astype(NPBF16),
            (eh + qh).astype(NPBF16), mask.astype(NPBF16),
            xe_bf, wconv.reshape(C, 9 * C),
        ], axis=1)
        in_maps.append({
            "pack": np.ascontiguousarray(pack),
            "xe": xe_bf,
            "xq": xq_bf,
            "gb": np.ascontiguousarray(np.concatenate([g, b], axis=1)),
        })
    return in_maps


def kernel(exemplar, query, W_lin, W_conv, gamma, beta):
    nc = _get_program()
    in_maps = _make_in_maps(exemplar, query, W_lin, W_conv, gamma, beta)
    res = bass_utils.run_bass_kernel_spmd(
        nc, in_maps, core_ids=list(range(N_CORES)), trace=False)
    out = np.empty((4, C, 64, 64), np.float32)
    for k in range(N_CORES):
        s, h = divmod(k, 2)
        out[s, :, 32 * h:32 * h + 32, :] = \
            res.results[k]["out"].reshape(C, 32, 64)
    return out


# revision 34
# speedup vs baseline: 4698.6437x; 4698.6437x over previous
"""Trainium2 Bass kernel for nn_CLM_26594437496868 (co-attention + conv/BN/leakyrelu).

Reference computation (b=4, c=64, h=w=64, hw=4096):
  EL = W_lin @ E                       # [c, hw] per sample
  A[n, m] = sum_c EL[c, n] Q[c, m]     # [hw, hw]
  query_c[c, n]    = sum_m Q[c, m] exp(A[n, m]) / sum_m exp(A[n, m])
  exemplar_c[c, n] = sum_m E[c, m] exp(A[m, n]) / sum_m exp(A[m, n])
  out_x = query_c + exemplar_c + E + Q
  y = conv3x3(out_x, W_conv); y = BN(y) * gamma + beta; leaky_relu(y, 0.1)

Sharding: 8 cores = 4 samples x 2 image-halves (rows 0-31 / 32-63).
Each core computes BOTH attention orientations for its 34-row slice
(rows R0-1 .. R0+32, one halo row each side, phantom rows zero-padded
by the host and masked out on device), the conv for all 64 output
channels of its 32 output rows, and local BN partial stats.  One tiny
AllGather ([64,2] fp32 per rank) combines BN stats across all 8 cores.
"""
import sys
if "/opt/trn_rl_repo" not in sys.path:
    sys.path.append("/opt/trn_rl_repo")

import numpy as np

import concourse.bass as bass
import concourse.bacc as bacc
import concourse.tile as tile
from concourse import mybir
from concourse import bass_utils

N_CORES = 8
C = 64                    # channels
HW = 4096                 # 64*64
W_IMG = 64
NH = 2176                 # 34 rows * 64 cols  (1 halo row each side)
NOUT = 2048               # 32 output rows * 64
N_BLOCKS = [(0, 512), (512, 512), (1024, 512), (1536, 512), (2048, 128)]
M_CHUNKS = 32             # 4096 / 128
BN_EPS = 1e-5
LEAKY = 0.1

BF16 = mybir.dt.bfloat16
F32 = mybir.dt.float32
NPBF16 = mybir.dt.np(BF16)

_COMPILED = None


def _build_program():
    nc = bacc.Bacc("TRN2", target_bir_lowering=False, debug=False,
                   enable_asserts=True, num_devices=N_CORES)

    # ---- I/O ----
    # pack (bf16, per-partition layout), attention-critical prefix first:
    #   [eh | xq | wt | qh | eqh | mask | xe | wconv(9*64)]
    PACKW = 4 * NH + 2 * HW + 9 * C + C
    PACK_CRIT = NH + HW + C
    d_pack = nc.dram_tensor("pack", [C, PACKW], BF16, kind="ExternalInput").ap()
    d_xe = nc.dram_tensor("xe", [C, HW], BF16, kind="ExternalInput").ap()
    d_xq = nc.dram_tensor("xq", [C, HW], BF16, kind="ExternalInput").ap()
    d_gb = nc.dram_tensor("gb", [C, 2], F32, kind="ExternalInput").ap()
    d_out = nc.dram_tensor("out", [C, NOUT], F32, kind="ExternalOutput").ap()

    from contextlib import ExitStack
    with tile.TileContext(nc) as tc, ExitStack() as ctx:
        consts = ctx.enter_context(tc.tile_pool(name="consts", bufs=1))
        big = ctx.enter_context(tc.tile_pool(name="big", bufs=1))
        expp = ctx.enter_context(tc.tile_pool(name="expp", bufs=3))
        smalls = ctx.enter_context(tc.tile_pool(name="smalls", bufs=2))
        dram = ctx.enter_context(tc.tile_pool(name="dram", bufs=1, space="DRAM"))
        ps_strip_cm = tc.tile_pool(name="ps_strip", bufs=2, space="PSUM")
        ps_strip = ps_strip_cm.__enter__()
        ps_pv_cm = tc.tile_pool(name="ps_pv", bufs=1, space="PSUM")
        ps_pv = ps_pv_cm.__enter__()

        # ---- load inputs: split packed DMA (critical prefix first) ----
        pack_sb = big.tile([C, PACKW], BF16)
        nc.sync.dma_start(out=pack_sb[:, 0:PACK_CRIT], in_=d_pack[:, 0:PACK_CRIT])
        nc.sync.dma_start(out=pack_sb[:, PACK_CRIT:], in_=d_pack[:, PACK_CRIT:])
        o0 = 0
        eh_sb = pack_sb[:, o0:o0 + NH]; o0 += NH
        xq_sb = pack_sb[:, o0:o0 + HW]; o0 += HW
        wt_sb = pack_sb[:, o0:o0 + C]; o0 += C
        qh_sb = pack_sb[:, o0:o0 + NH]; o0 += NH
        eqh_sb = pack_sb[:, o0:o0 + NH]; o0 += NH
        mask_sb = pack_sb[:, o0:o0 + NH]; o0 += NH
        xe_sb = pack_sb[:, o0:o0 + HW]; o0 += HW
        wconv_sb = pack_sb[:, o0:o0 + 9 * C].rearrange(
            "p (t o) -> p t o", t=9); o0 += 9 * C
        gb_sb = consts.tile([C, 2], F32)
        nc.sync.dma_start(out=gb_sb[:], in_=d_gb[:])
        gamma_sb = gb_sb[:, 0:1]
        beta_sb = gb_sb[:, 1:2]

        alpha_sb = consts.tile([C, 1], F32)
        nc.vector.memset(alpha_sb[:], LEAKY)
        eps_sb = consts.tile([C, 1], F32)
        nc.vector.memset(eps_sb[:], BN_EPS)
        # warm the ACT exp table while the input DMAs run
        warm_sb = consts.tile([C, 1], F32)
        nc.scalar.activation(out=warm_sb[:], in_=eps_sb[:],
                             func=mybir.ActivationFunctionType.Exp)

        # ---- EL-half = W_lin @ eh (gates the first attention strip) ----
        elf_sb = big.tile([C, HW], BF16)        # full-m EL for orientation 2 lhsT
        elh_sb = big.tile([C, NH], BF16)        # half-n EL for orientation 1 rhs
        for (off, nb) in N_BLOCKS:
            ps_el = ps_strip.tile([128, 3, 512], F32, tag="sp")
            nc.tensor.matmul(ps_el[0:C, 0, 0:nb], wt_sb[:],
                             eh_sb[:, off:off + nb], start=True, stop=True)
            nc.vector.tensor_copy(elh_sb[:, off:off + nb], ps_el[0:C, 0, 0:nb])

        # [Q^T | 1] via xbar DMA transpose straight from DRAM
        qtd = big.tile([128, M_CHUNKS, C], BF16)
        etd = big.tile([128, M_CHUNKS, C], BF16)
        nc.sync.dma_start_transpose(out=qtd[:], in_=d_xq[:])
        qt_sb = big.tile([128, M_CHUNKS, C + 1], BF16)
        et_sb = big.tile([128, M_CHUNKS, C + 1], BF16)
        nc.vector.memset(qt_sb[:, :, C:C + 1], 1.0)
        nc.vector.tensor_copy(qt_sb[:, :, 0:C], qtd[:])

        def emit_o1_prep():
            # orientation-1 operands; emitted inside the o=0 window of block 0.
            # Their PSUM lives in the (idle during o=0) pv1 slot.
            for j in range(HW // 512):
                ps_el = ps_pv.tile([C + 1, 512], F32, tag="pv1")
                nc.tensor.matmul(ps_el[0:C, :], wt_sb[:],
                                 xe_sb[:, j * 512:(j + 1) * 512],
                                 start=True, stop=True)
                nc.vector.tensor_copy(elf_sb[:, j * 512:(j + 1) * 512],
                                      ps_el[0:C, :])
            nc.sync.dma_start_transpose(out=etd[:], in_=d_xe[:])
            nc.vector.memset(et_sb[:, :, C:C + 1], 1.0)
            nc.vector.tensor_copy(et_sb[:, :, 0:C], etd[:])

        # ---- conv input (built incrementally): [64, 34 rows, 66 cols] ----
        xpad = big.tile([C, 34, 66], BF16)
        nc.vector.memset(xpad[:], 0.0)

        # ---- attention: both orientations, streamed over m in 3-chunk strips ----
        s_lhs = (xq_sb, elf_sb)       # T1[m,l] = sum_c Q[c,m] ELh[c,l] ; T2 = sum_c EL[c,m] qh[c,l]
        s_rhs = (elh_sb, qh_sb)
        pv_lhs = (qt_sb, et_sb)
        STRIPS = [(0, 3), (3, 3), (6, 3), (9, 3), (12, 3), (15, 3), (18, 3),
                  (21, 3), (24, 3), (27, 3), (30, 2)]  # (chunk0, nchunks)
        y_sb = big.tile([C, NOUT], F32)
        st = smalls.tile([C, 4, 6], F32, tag="st")

        def emit_conv_block(rb, pool):
            # conv out rows 8rb..8rb+7 <- xpad rows 8rb..8rb+9
            yp = pool.tile([C + 1, 512], F32, tag="yp")
            for tap in range(9):
                dy, dx = tap // 3, tap % 3
                nc.tensor.matmul(
                    yp[0:C, :],
                    wconv_sb[:, tap, :],
                    xpad[:, 8 * rb + dy:8 * rb + dy + 8, dx:dx + 64],
                    start=(tap == 0), stop=(tap == 8))
            nc.vector.tensor_copy(y_sb[:, rb * 512:(rb + 1) * 512], yp[0:C, :])
            nc.vector.bn_stats(out=st[:, rb, :],
                               in_=y_sb[:, rb * 512:(rb + 1) * 512])

        for ib, (off, nb) in enumerate(N_BLOCKS):
            nrows = nb // W_IMG
            r0 = off // W_IMG
            pvs = []
            for o in (0, 1):
                pv = ps_pv.tile([C + 1, 512], F32, tag=f"pv{o}")
                pvs.append(pv)
                for (c0, ns) in STRIPS:
                    sp = ps_strip.tile([128, 3, 512], F32, tag="sp")
                    for u in range(ns):
                        j = c0 + u
                        nc.tensor.matmul(sp[:, u, 0:nb],
                                         s_lhs[o][:, 128 * j:128 * j + 128],
                                         s_rhs[o][:, off:off + nb],
                                         start=True, stop=True)
                    ex = expp.tile([128, 3, 512], BF16, tag="ex")
                    nc.scalar.activation(out=ex[:, 0:ns, 0:nb],
                                         in_=sp[:, 0:ns, 0:nb],
                                         func=mybir.ActivationFunctionType.Exp)
                    for u in range(ns):
                        j = c0 + u
                        nc.tensor.matmul(pv[:, 0:nb], pv_lhs[o][:, j, :],
                                         ex[:, u, 0:nb],
                                         start=(j == 0), stop=(j == M_CHUNKS - 1))
                if off == 0 and o == 0:
                    emit_o1_prep()
            # normalize:  O[c, l] * (1/D[l]) ; 1/D broadcast via gpsimd.
            # Copy PSUM->SBUF first so the accumulator bank frees quickly.
            zs = []
            for o in (0, 1):
                pvc = smalls.tile([C + 1, 512], F32, tag=f"pvc{o}")
                nc.vector.tensor_copy(pvc[:, 0:nb], pvs[o][:, 0:nb])
                rd = smalls.tile([128, 512], F32, tag="rd")
                nc.vector.reciprocal(rd[64:65, 0:nb], pvc[C:C + 1, 0:nb])
                rd0 = smalls.tile([1, 512], F32, tag="rd0")
                nc.sync.dma_start(out=rd0[0:1, 0:nb], in_=rd[64:65, 0:nb])
                bc = smalls.tile([C, 512], F32, tag="bc")
                nc.gpsimd.partition_broadcast(bc[:, 0:nb], rd0[0:1, 0:nb])
                z = smalls.tile([C, 512], F32, tag=f"z{o}")
                nc.vector.tensor_mul(z[:, 0:nb], pvc[0:C, 0:nb], bc[:, 0:nb])
                zs.append(z)
            zsum = smalls.tile([C, 512], F32, tag="zsum")
            nc.vector.tensor_add(zsum[:, 0:nb], zs[0][:, 0:nb], zs[1][:, 0:nb])
            nc.vector.tensor_add(zsum[:, 0:nb], zsum[:, 0:nb],
                                 eqh_sb[:, off:off + nb])
            # masked cast into the padded conv input
            nc.vector.scalar_tensor_tensor(
                out=xpad[:, r0:r0 + nrows, 1:65],
                in0=zsum[:, 0:nb].rearrange("p (r w) -> p r w", w=W_IMG),
                scalar=1.0,
                in1=mask_sb[:, off:off + nb].rearrange("p (r w) -> p r w", w=W_IMG),
                op0=mybir.AluOpType.mult,
                op1=mybir.AluOpType.mult,
            )
        # ---- conv 3x3 + BN stats (after attention; PSUM pools swapped) ----
        ps_pv_cm.__exit__(None, None, None)
        ps_strip_cm.__exit__(None, None, None)
        ps_conv = ctx.enter_context(tc.tile_pool(name="ps_conv", bufs=2, space="PSUM"))
        for rb in range(4):
            emit_conv_block(rb, ps_conv)
        mv = smalls.tile([C, 2], F32, tag="mv")
        nc.vector.bn_aggr(out=mv[:], in_=st[:])

        # ---- BN stats AllGather (cheaper than AllReduce) + local reduce ----
        ccs = smalls.tile([C, 2], F32, tag="ccs")
        m2 = smalls.tile([C, 1], F32, tag="m2")
        nc.vector.tensor_mul(m2[:], mv[:, 0:1], mv[:, 0:1])
        nc.vector.tensor_copy(ccs[:, 0:1], mv[:, 0:1])
        nc.vector.tensor_add(ccs[:, 1:2], mv[:, 1:2], m2[:])
        cc_in = dram.tile([C, 2], F32)
        cc_out = dram.tile([N_CORES, C, 2], F32, addr_space="Shared")
        nc.sync.dma_start(out=cc_in[:], in_=ccs[:])
        nc.gpsimd.collective_compute(
            "AllGather", mybir.AluOpType.bypass,
            replica_groups=[list(range(N_CORES))],
            ins=[cc_in.opt()], outs=[cc_out.opt()])
        gath = smalls.tile([C, 2, N_CORES], F32, tag="gath")
        nc.sync.dma_start(out=gath[:],
                          in_=cc_out[:].rearrange("r c v -> c v r"))
        red = smalls.tile([C, 2], F32, tag="red")
        nc.vector.tensor_reduce(red[:], gath[:], axis=mybir.AxisListType.X,
                                op=mybir.AluOpType.add)

        # mu = red0/8 ; var = red1/8 - mu^2 ; rstd = exp(-0.5*ln(var+eps))
        nc.vector.tensor_scalar_mul(red[:], red[:], 1.0 / N_CORES)
        mu = red[:, 0:1]
        var = smalls.tile([C, 1], F32, tag="var")
        mu2 = smalls.tile([C, 1], F32, tag="mu2")
        nc.vector.tensor_mul(mu2[:], mu, mu)
        nc.vector.tensor_sub(var[:], red[:, 1:2], mu2[:])
        lnv = smalls.tile([C, 1], F32, tag="lnv")
        nc.scalar.activation(out=lnv[:], in_=var[:],
                             func=mybir.ActivationFunctionType.Ln, bias=eps_sb[:])
        rstd = smalls.tile([C, 1], F32, tag="rstd")
        nc.scalar.activation(out=rstd[:], in_=lnv[:],
                             func=mybir.ActivationFunctionType.Exp, scale=-0.5)
        scale_f = smalls.tile([C, 1], F32, tag="scale_f")
        bias_f = smalls.tile([C, 1], F32, tag="bias_f")
        nc.vector.tensor_mul(scale_f[:], gamma_sb[:], rstd[:])
        nc.vector.tensor_mul(bias_f[:], mu, scale_f[:])
        nc.vector.tensor_sub(bias_f[:], beta_sb[:], bias_f[:])

        # ---- apply BN + leaky relu, write out ----
        # Prelu respects a per-partition alpha AP (Lrelu ignores its alpha
        # and uses the hardware default 0.01) -> one ACT op per block.
        osb = big.tile([C, NOUT], F32)
        nc.scalar.activation(out=osb[:], in_=y_sb[:],
                             func=mybir.ActivationFunctionType.Prelu,
                             bias=bias_f[:], scale=scale_f[:],
                             alpha=alpha_sb[:])
        nc.sync.dma_start(out=d_out[:], in_=osb[:])

    nc.compile()
    return nc


def _get_program():
    global _COMPILED
    if _COMPILED is None:
        _COMPILED = _build_program()
    return _COMPILED


def _make_in_maps(exemplar, query, W_lin, W_conv, gamma, beta):
    E = np.asarray(exemplar, dtype=np.float32).reshape(4, C, HW)
    Q = np.asarray(query, dtype=np.float32).reshape(4, C, HW)
    wt = np.ascontiguousarray(np.asarray(W_lin, np.float32).T).astype(NPBF16)
    assert wt.shape == (C, C)
    wconv = np.ascontiguousarray(
        np.asarray(W_conv, np.float32).transpose(1, 2, 3, 0).reshape(C, 9, C)
    ).astype(NPBF16)
    g = np.asarray(gamma, np.float32).reshape(C, 1)
    b = np.asarray(beta, np.float32).reshape(C, 1)

    zeros = np.zeros((C, W_IMG), np.float32)
    in_maps = []
    for k in range(N_CORES):
        s, h = divmod(k, 2)
        if h == 0:
            sl = lambda X: np.concatenate([zeros, X[s][:, :NH - W_IMG]], axis=1)
        else:
            sl = lambda X: np.concatenate([X[s][:, HW - (NH - W_IMG):], zeros], axis=1)
        eh = sl(E)
        qh = sl(Q)
        mask = np.ones((C, NH), np.float32)
        if h == 0:
            mask[:, :W_IMG] = 0.0
        else:
            mask[:, NH - W_IMG:] = 0.0
        xe_bf = E[s].astype(NPBF16)
        xq_bf = Q[s].astype(NPBF16)
        # order must match the device-side unpack:
        #   [eh | xq | wt | qh | eqh | mask | xe | wconv]
        pack = np.concatenate([
            eh.astype(NPBF16), xq_bf, wt, qh.astype(NPBF16),
            (eh + qh).astype(NPBF16), mask.astype(NPBF16),
            xe_bf, wconv.reshape(C, 9 * C),
        ], axis=1)
        in_maps.append({
            "pack": np.ascontiguousarray(pack),
            "xe": xe_bf,
            "xq": xq_bf,
            "gb": np.ascontiguousarray(np.concatenate([g, b], axis=1)),
        })
    return in_maps


def kernel(exemplar, query, W_lin, W_conv, gamma, beta):
    nc = _get_program()
    in_maps = _make_in_maps(exemplar, query, W_lin, W_conv, gamma, beta)
    res = bass_utils.run_bass_kernel_spmd(
        nc, in_maps, core_ids=list(range(N_CORES)), trace=False)
    out = np.empty((4, C, 64, 64), np.float32)
    for k in range(N_CORES):
        s, h = divmod(k, 2)
        out[s, :, 32 * h:32 * h + 32, :] = \
            res.results[k]["out"].reshape(C, 32, 64)
    return out


# revision 37
# speedup vs baseline: 4795.0294x; 1.0205x over previous
"""Trainium2 Bass kernel for nn_CLM_26594437496868 (co-attention + conv/BN/leakyrelu).

Reference computation (b=4, c=64, h=w=64, hw=4096):
  EL = W_lin @ E                       # [c, hw] per sample
  A[n, m] = sum_c EL[c, n] Q[c, m]     # [hw, hw]
  query_c[c, n]    = sum_m Q[c, m] exp(A[n, m]) / sum_m exp(A[n, m])
  exemplar_c[c, n] = sum_m E[c, m] exp(A[m, n]) / sum_m exp(A[m, n])
  out_x = query_c + exemplar_c + E + Q
  y = conv3x3(out_x, W_conv); y = BN(y) * gamma + beta; leaky_relu(y, 0.1)

Sharding: 8 cores = 4 samples x 2 image-halves (rows 0-31 / 32-63).
Each core computes BOTH attention orientations for its 34-row slice
(rows R0-1 .. R0+32, one halo row each side, phantom rows zero-padded
by the host and masked out on device), the conv for all 64 output
channels of its 32 output rows, and local BN partial stats.  One tiny
AllGather ([64,2] fp32 per rank) combines BN stats across all 8 cores.
"""
import sys
if "/opt/trn_rl_repo" not in sys.path:
    sys.path.append("/opt/trn_rl_repo")

import numpy as np

import concourse.bass as bass
import concourse.bacc as bacc
import concourse.tile as tile
from concourse import mybir
from concourse import bass_utils

N_CORES = 8
C = 64                    # channels
HW = 4096                 # 64*64
W_IMG = 64
NH = 2176                 # 34 rows * 64 cols  (1 halo row each side)
NOUT = 2048               # 32 output rows * 64
N_BLOCKS = [(0, 512), (512, 512), (1024, 512), (1536, 512), (2048, 128)]
M_CHUNKS = 32             # 4096 / 128
BN_EPS = 1e-5
LEAKY = 0.1

BF16 = mybir.dt.bfloat16
F32 = mybir.dt.float32
NPBF16 = mybir.dt.np(BF16)

_COMPILED = None


def _build_program():
    nc = bacc.Bacc("TRN2", target_bir_lowering=False, debug=False,
                   enable_asserts=True, num_devices=N_CORES)

    # ---- I/O ----
    # pack (bf16, per-partition layout), attention-critical prefix first:
    #   [eh | xq | wt | qh | eqh | mask | xe | wconv(9*64)]
    PACKW = 4 * NH + 2 * HW + 9 * C + C
    PACK_CRIT = NH + HW + C
    d_pack = nc.dram_tensor("pack", [C, PACKW], BF16, kind="ExternalInput").ap()
    d_xe = nc.dram_tensor("xe", [C, HW], BF16, kind="ExternalInput").ap()
    d_xq = nc.dram_tensor("xq", [C, HW], BF16, kind="ExternalInput").ap()
    d_gb = nc.dram_tensor("gb", [C, 2], F32, kind="ExternalInput").ap()
    d_out = nc.dram_tensor("out", [C, NOUT], F32, kind="ExternalOutput").ap()

    from contextlib import ExitStack
    with tile.TileContext(nc) as tc, ExitStack() as ctx:
        consts = ctx.enter_context(tc.tile_pool(name="consts", bufs=1))
        big = ctx.enter_context(tc.tile_pool(name="big", bufs=1))
        expp = ctx.enter_context(tc.tile_pool(name="expp", bufs=3))
        smalls = ctx.enter_context(tc.tile_pool(name="smalls", bufs=2))
        dram = ctx.enter_context(tc.tile_pool(name="dram", bufs=1, space="DRAM"))
        ps_strip_cm = tc.tile_pool(name="ps_strip", bufs=2, space="PSUM")
        ps_strip = ps_strip_cm.__enter__()
        ps_pv_cm = tc.tile_pool(name="ps_pv", bufs=1, space="PSUM")
        ps_pv = ps_pv_cm.__enter__()

        # ---- load inputs: split packed DMA (critical prefix first) ----
        pack_sb = big.tile([C, PACKW], BF16)
        nc.sync.dma_start(out=pack_sb[:, 0:PACK_CRIT], in_=d_pack[:, 0:PACK_CRIT])
        nc.sync.dma_start(out=pack_sb[:, PACK_CRIT:], in_=d_pack[:, PACK_CRIT:])
        o0 = 0
        eh_sb = pack_sb[:, o0:o0 + NH]; o0 += NH
        xq_sb = pack_sb[:, o0:o0 + HW]; o0 += HW
        wt_sb = pack_sb[:, o0:o0 + C]; o0 += C
        qh_sb = pack_sb[:, o0:o0 + NH]; o0 += NH
        eqh_sb = pack_sb[:, o0:o0 + NH]; o0 += NH
        mask_sb = pack_sb[:, o0:o0 + NH]; o0 += NH
        xe_sb = pack_sb[:, o0:o0 + HW]; o0 += HW
        wconv_sb = pack_sb[:, o0:o0 + 9 * C].rearrange(
            "p (t o) -> p t o", t=9); o0 += 9 * C
        gb_sb = consts.tile([C, 2], F32)
        nc.sync.dma_start(out=gb_sb[:], in_=d_gb[:])
        gamma_sb = gb_sb[:, 0:1]
        beta_sb = gb_sb[:, 1:2]

        alpha_sb = consts.tile([C, 1], F32)
        nc.vector.memset(alpha_sb[:], LEAKY)
        eps_sb = consts.tile([C, 1], F32)
        nc.vector.memset(eps_sb[:], BN_EPS)
        # warm the ACT exp table while the input DMAs run
        warm_sb = consts.tile([C, 1], F32)
        nc.scalar.activation(out=warm_sb[:], in_=eps_sb[:],
                             func=mybir.ActivationFunctionType.Exp)

        # ---- EL-half = W_lin @ eh (gates the first attention strip) ----
        elf_sb = big.tile([C, HW], BF16)        # full-m EL for orientation 2 lhsT
        elh_sb = big.tile([C, NH], BF16)        # half-n EL for orientation 1 rhs
        for (off, nb) in N_BLOCKS:
            ps_el = ps_strip.tile([128, 3, 512], F32, tag="sp")
            nc.tensor.matmul(ps_el[0:C, 0, 0:nb], wt_sb[:],
                             eh_sb[:, off:off + nb], start=True, stop=True)
            nc.vector.tensor_copy(elh_sb[:, off:off + nb], ps_el[0:C, 0, 0:nb])

        # [Q^T | 1] via xbar DMA transpose straight from DRAM
        qtd = big.tile([128, M_CHUNKS, C], BF16)
        etd = big.tile([128, M_CHUNKS, C], BF16)
        nc.sync.dma_start_transpose(out=qtd[:], in_=d_xq[:])
        qt_sb = big.tile([128, M_CHUNKS, C + 1], BF16)
        et_sb = big.tile([128, M_CHUNKS, C + 1], BF16)
        nc.vector.memset(qt_sb[:, :, C:C + 1], 1.0)
        nc.vector.tensor_copy(qt_sb[:, :, 0:C], qtd[:])

        def emit_o1_prep():
            # orientation-1 operands; emitted inside the o=0 window of block 0.
            # Their PSUM lives in the (idle during o=0) pv1 slot.
            for j in range(HW // 512):
                ps_el = ps_pv.tile([C + 1, 512], F32, tag="pv1")
                nc.tensor.matmul(ps_el[0:C, :], wt_sb[:],
                                 xe_sb[:, j * 512:(j + 1) * 512],
                                 start=True, stop=True)
                nc.vector.tensor_copy(elf_sb[:, j * 512:(j + 1) * 512],
                                      ps_el[0:C, :])
            nc.sync.dma_start_transpose(out=etd[:], in_=d_xe[:])
            nc.vector.memset(et_sb[:, :, C:C + 1], 1.0)
            nc.vector.tensor_copy(et_sb[:, :, 0:C], etd[:])

        # ---- conv input (built incrementally): [64, 34 rows, 66 cols] ----
        xpad = big.tile([C, 34, 66], BF16)
        nc.vector.memset(xpad[:], 0.0)

        # ---- attention: both orientations, streamed over m in 3-chunk strips ----
        s_lhs = (xq_sb, elf_sb)       # T1[m,l] = sum_c Q[c,m] ELh[c,l] ; T2 = sum_c EL[c,m] qh[c,l]
        s_rhs = (elh_sb, qh_sb)
        pv_lhs = (qt_sb, et_sb)
        # (chunk0, nchunks) strips; chunk width matches the n-block width so
        # one strip tile (3 PSUM banks) always holds nchunks * nb <= 1536 elems
        STRIPS_512 = [(0, 3), (3, 3), (6, 3), (9, 3), (12, 3), (15, 3), (18, 3),
                      (21, 3), (24, 3), (27, 3), (30, 2)]
        STRIPS_128 = [(0, 12), (12, 12), (24, 8)]
        y_sb = big.tile([C, NOUT], F32)
        st = smalls.tile([C, 4, 6], F32, tag="st")

        def emit_conv_block(rb, pool):
            # conv out rows 8rb..8rb+7 <- xpad rows 8rb..8rb+9
            yp = pool.tile([C + 1, 512], F32, tag="yp")
            for tap in range(9):
                dy, dx = tap // 3, tap % 3
                nc.tensor.matmul(
                    yp[0:C, :],
                    wconv_sb[:, tap, :],
                    xpad[:, 8 * rb + dy:8 * rb + dy + 8, dx:dx + 64],
                    start=(tap == 0), stop=(tap == 8))
            nc.vector.tensor_copy(y_sb[:, rb * 512:(rb + 1) * 512], yp[0:C, :])
            nc.vector.bn_stats(out=st[:, rb, :],
                               in_=y_sb[:, rb * 512:(rb + 1) * 512])

        for ib, (off, nb) in enumerate(N_BLOCKS):
            nrows = nb // W_IMG
            r0 = off // W_IMG
            strips = STRIPS_512 if nb == 512 else STRIPS_128
            nsub = 3 if nb == 512 else 12
            pvs = []
            for o in (0, 1):
                pv = ps_pv.tile([C + 1, 512], F32, tag=f"pv{o}")
                pvs.append(pv)
                for (c0, ns) in strips:
                    sp_flat = ps_strip.tile([128, 1536], F32, tag="sp")
                    sp = sp_flat.rearrange("p (a b) -> p a b", b=nb)
                    for u in range(ns):
                        j = c0 + u
                        nc.tensor.matmul(sp[:, u, :],
                                         s_lhs[o][:, 128 * j:128 * j + 128],
                                         s_rhs[o][:, off:off + nb],
                                         start=True, stop=True)
                    ex_flat = expp.tile([128, 1536], BF16, tag="ex")
                    ex = ex_flat.rearrange("p (a b) -> p a b", b=nb)
                    nc.scalar.activation(out=ex[:, 0:ns, :],
                                         in_=sp[:, 0:ns, :],
                                         func=mybir.ActivationFunctionType.Exp)
                    for u in range(ns):
                        j = c0 + u
                        nc.tensor.matmul(pv[:, 0:nb], pv_lhs[o][:, j, :],
                                         ex[:, u, :],
                                         start=(j == 0), stop=(j == M_CHUNKS - 1))
                if off == 0 and o == 0:
                    emit_o1_prep()
            # normalize:  O[c, l] * (1/D[l]) ; 1/D broadcast via gpsimd.
            # Copy PSUM->SBUF first so the accumulator bank frees quickly.
            zs = []
            for o in (0, 1):
                pvc = smalls.tile([C + 1, 512], F32, tag=f"pvc{o}")
                nc.vector.tensor_copy(pvc[:, 0:nb], pvs[o][:, 0:nb])
                rd = smalls.tile([128, 512], F32, tag="rd")
                nc.vector.reciprocal(rd[64:65, 0:nb], pvc[C:C + 1, 0:nb])
                rd0 = smalls.tile([1, 512], F32, tag="rd0")
                nc.sync.dma_start(out=rd0[0:1, 0:nb], in_=rd[64:65, 0:nb])
                bc = smalls.tile([C, 512], F32, tag="bc")
                nc.gpsimd.partition_broadcast(bc[:, 0:nb], rd0[0:1, 0:nb])
                z = smalls.tile([C, 512], F32, tag=f"z{o}")
                nc.vector.tensor_mul(z[:, 0:nb], pvc[0:C, 0:nb], bc[:, 0:nb])
                zs.append(z)
            zsum = smalls.tile([C, 512], F32, tag="zsum")
            nc.vector.tensor_add(zsum[:, 0:nb], zs[0][:, 0:nb], zs[1][:, 0:nb])
            nc.vector.tensor_add(zsum[:, 0:nb], zsum[:, 0:nb],
                                 eqh_sb[:, off:off + nb])
            # masked cast into the padded conv input
            nc.vector.scalar_tensor_tensor(
                out=xpad[:, r0:r0 + nrows, 1:65],
                in0=zsum[:, 0:nb].rearrange("p (r w) -> p r w", w=W_IMG),
                scalar=1.0,
                in1=mask_sb[:, off:off + nb].rearrange("p (r w) -> p r w", w=W_IMG),
                op0=mybir.AluOpType.mult,
                op1=mybir.AluOpType.mult,
            )
        # ---- conv 3x3 + BN stats (after attention; PSUM pools swapped) ----
        ps_pv_cm.__exit__(None, None, None)
        ps_strip_cm.__exit__(None, None, None)
        ps_conv = ctx.enter_context(tc.tile_pool(name="ps_conv", bufs=2, space="PSUM"))
        for rb in range(4):
            emit_conv_block(rb, ps_conv)
        mv = smalls.tile([C, 2], F32, tag="mv")
        nc.vector.bn_aggr(out=mv[:], in_=st[:])

        # ---- BN stats AllGather (cheaper than AllReduce) + local reduce ----
        ccs = smalls.tile([C, 2], F32, tag="ccs")
        m2 = smalls.tile([C, 1], F32, tag="m2")
        nc.vector.tensor_mul(m2[:], mv[:, 0:1], mv[:, 0:1])
        nc.vector.tensor_copy(ccs[:, 0:1], mv[:, 0:1])
        nc.vector.tensor_add(ccs[:, 1:2], mv[:, 1:2], m2[:])
        cc_in = dram.tile([C, 2], F32)
        cc_out = dram.tile([N_CORES, C, 2], F32, addr_space="Shared")
        nc.sync.dma_start(out=cc_in[:], in_=ccs[:])
        nc.gpsimd.collective_compute(
            "AllGather", mybir.AluOpType.bypass,
            replica_groups=[list(range(N_CORES))],
            ins=[cc_in.opt()], outs=[cc_out.opt()])
        gath = smalls.tile([C, 2, N_CORES], F32, tag="gath")
        nc.sync.dma_start(out=gath[:],
                          in_=cc_out[:].rearrange("r c v -> c v r"))
        red = smalls.tile([C, 2], F32, tag="red")
        nc.vector.tensor_reduce(red[:], gath[:], axis=mybir.AxisListType.X,
                                op=mybir.AluOpType.add)

        # mu = red0/8 ; var = red1/8 - mu^2 ; rstd = exp(-0.5*ln(var+eps))
        nc.vector.tensor_scalar_mul(red[:], red[:], 1.0 / N_CORES)
        mu = red[:, 0:1]
        var = smalls.tile([C, 1], F32, tag="var")
        mu2 = smalls.tile([C, 1], F32, tag="mu2")
        nc.vector.tensor_mul(mu2[:], mu, mu)
        nc.vector.tensor_sub(var[:], red[:, 1:2], mu2[:])
        lnv = smalls.tile([C, 1], F32, tag="lnv")
        nc.scalar.activation(out=lnv[:], in_=var[:],
                             func=mybir.ActivationFunctionType.Ln, bias=eps_sb[:])
        rstd = smalls.tile([C, 1], F32, tag="rstd")
        nc.scalar.activation(out=rstd[:], in_=lnv[:],
                             func=mybir.ActivationFunctionType.Exp, scale=-0.5)
        scale_f = smalls.tile([C, 1], F32, tag="scale_f")
        bias_f = smalls.tile([C, 1], F32, tag="bias_f")
        nc.vector.tensor_mul(scale_f[:], gamma_sb[:], rstd[:])
        nc.vector.tensor_mul(bias_f[:], mu, scale_f[:])
        nc.vector.tensor_sub(bias_f[:], beta_sb[:], bias_f[:])

        # ---- apply BN + leaky relu, write out ----
        # Prelu respects a per-partition alpha AP (Lrelu ignores its alpha
        # and uses the hardware default 0.01) -> one ACT op per block.
        osb = big.tile([C, NOUT], F32)
        nc.scalar.activation(out=osb[:], in_=y_sb[:],
                             func=mybir.ActivationFunctionType.Prelu,
                             bias=bias_f[:], scale=scale_f[:],
                             alpha=alpha_sb[:])
        nc.sync.dma_start(out=d_out[:], in_=osb[:])

    nc.compile()
    return nc


def _get_program():
    global _COMPILED
    if _COMPILED is None:
        _COMPILED = _build_program()
    return _COMPILED


def _make_in_maps(exemplar, query, W_lin, W_conv, gamma, beta):
    E = np.asarray(exemplar, dtype=np.float32).reshape(4, C, HW)
    Q = np.asarray(query, dtype=np.float32).reshape(4, C, HW)
    wt = np.ascontiguousarray(np.asarray(W_lin, np.float32).T).astype(NPBF16)
    assert wt.shape == (C, C)
    wconv = np.ascontiguousarray(
        np.asarray(W_conv, np.float32).transpose(1, 2, 3, 0).reshape(C, 9, C)
    ).astype(NPBF16)
    g = np.asarray(gamma, np.float32).reshape(C, 1)
    b = np.asarray(beta, np.float32).reshape(C, 1)

    zeros = np.zeros((C, W_IMG), np.float32)
    in_maps = []
    for k in range(N_CORES):
        s, h = divmod(k, 2)
        if h == 0:
            sl = lambda X: np.concatenate([zeros, X[s][:, :NH - W_IMG]], axis=1)
        else:
            sl = lambda X: np.concatenate([X[s][:, HW - (NH - W_IMG):], zeros], axis=1)
        eh = sl(E)
        qh = sl(Q)
        mask = np.ones((C, NH), np.float32)
        if h == 0:
            mask[:, :W_IMG] = 0.0
        else:
            mask[:, NH - W_IMG:] = 0.0
        xe_bf = E[s].astype(NPBF16)
        xq_bf = Q[s].astype(NPBF16)
        # order must match the device-side unpack:
        #   [eh | xq | wt | qh | eqh | mask | xe | wconv]
        pack = np.concatenate([
            eh.astype(NPBF16), xq_bf, wt, qh.astype(NPBF16),
            (eh + qh).astype(NPBF16), mask.astype(NPBF16),
            xe_bf, wconv.reshape(C, 9 * C),
        ], axis=1)
        in_maps.append({
            "pack": np.ascontiguousarray(pack),
            "xe": xe_bf,
            "xq": xq_bf,
            "gb": np.ascontiguousarray(np.concatenate([g, b], axis=1)),
        })
    return in_maps


def kernel(exemplar, query, W_lin, W_conv, gamma, beta):
    nc = _get_program()
    in_maps = _make_in_maps(exemplar, query, W_lin, W_conv, gamma, beta)
    res = bass_utils.run_bass_kernel_spmd(
        nc, in_maps, core_ids=list(range(N_CORES)), trace=False)
    out = np.empty((4, C, 64, 64), np.float32)
    for k in range(N_CORES):
        s, h = divmod(k, 2)
        out[s, :, 32 * h:32 * h + 32, :] = \
            res.results[k]["out"].reshape(C, 32, 64)
    return out


# revision 38
# speedup vs baseline: 4824.8538x; 1.0062x over previous
"""Trainium2 Bass kernel for nn_CLM_26594437496868 (co-attention + conv/BN/leakyrelu).

Reference computation (b=4, c=64, h=w=64, hw=4096):
  EL = W_lin @ E                       # [c, hw] per sample
  A[n, m] = sum_c EL[c, n] Q[c, m]     # [hw, hw]
  query_c[c, n]    = sum_m Q[c, m] exp(A[n, m]) / sum_m exp(A[n, m])
  exemplar_c[c, n] = sum_m E[c, m] exp(A[m, n]) / sum_m exp(A[m, n])
  out_x = query_c + exemplar_c + E + Q
  y = conv3x3(out_x, W_conv); y = BN(y) * gamma + beta; leaky_relu(y, 0.1)

Sharding: 8 cores = 4 samples x 2 image-halves (rows 0-31 / 32-63).
Each core computes BOTH attention orientations for its 34-row slice
(rows R0-1 .. R0+32, one halo row each side, phantom rows zero-padded
by the host and masked out on device), the conv for all 64 output
channels of its 32 output rows, and local BN partial stats.  One tiny
AllGather ([64,2] fp32 per rank) combines BN stats across all 8 cores.
"""
import sys
if "/opt/trn_rl_repo" not in sys.path:
    sys.path.append("/opt/trn_rl_repo")

import numpy as np

import concourse.bass as bass
import concourse.bacc as bacc
import concourse.tile as tile
from concourse import mybir
from concourse import bass_utils

N_CORES = 8
C = 64                    # channels
HW = 4096                 # 64*64
W_IMG = 64
NH = 2176                 # 34 rows * 64 cols  (1 halo row each side)
NOUT = 2048               # 32 output rows * 64
N_BLOCKS = [(0, 512), (512, 512), (1024, 512), (1536, 512), (2048, 128)]
M_CHUNKS = 32             # 4096 / 128
BN_EPS = 1e-5
LEAKY = 0.1

BF16 = mybir.dt.bfloat16
F32 = mybir.dt.float32
NPBF16 = mybir.dt.np(BF16)

_COMPILED = None


def _build_program():
    nc = bacc.Bacc("TRN2", target_bir_lowering=False, debug=False,
                   enable_asserts=True, num_devices=N_CORES)

    # ---- I/O ----
    # pack (bf16, per-partition layout), attention-critical prefix first:
    #   [eh | xq | wt | qh | eqh | mask | xe | wconv(9*64)]
    PACKW = 4 * NH + 2 * HW + 9 * C + C
    PACK_CRIT = NH + HW + C
    d_pack = nc.dram_tensor("pack", [C, PACKW], BF16, kind="ExternalInput").ap()
    d_xe = nc.dram_tensor("xe", [C, HW], BF16, kind="ExternalInput").ap()
    d_xq = nc.dram_tensor("xq", [C, HW], BF16, kind="ExternalInput").ap()
    d_gb = nc.dram_tensor("gb", [C, 2], F32, kind="ExternalInput").ap()
    d_out = nc.dram_tensor("out", [C, NOUT], F32, kind="ExternalOutput").ap()

    from contextlib import ExitStack
    with tile.TileContext(nc) as tc, ExitStack() as ctx:
        consts = ctx.enter_context(tc.tile_pool(name="consts", bufs=1))
        big = ctx.enter_context(tc.tile_pool(name="big", bufs=1))
        expp = ctx.enter_context(tc.tile_pool(name="expp", bufs=4))
        smalls = ctx.enter_context(tc.tile_pool(name="smalls", bufs=3))
        dram = ctx.enter_context(tc.tile_pool(name="dram", bufs=1, space="DRAM"))
        ps_strip_cm = tc.tile_pool(name="ps_strip", bufs=2, space="PSUM")
        ps_strip = ps_strip_cm.__enter__()
        ps_pv_cm = tc.tile_pool(name="ps_pv", bufs=1, space="PSUM")
        ps_pv = ps_pv_cm.__enter__()

        # ---- load inputs: split packed DMA (critical prefix first) ----
        pack_sb = big.tile([C, PACKW], BF16)
        nc.sync.dma_start(out=pack_sb[:, 0:PACK_CRIT], in_=d_pack[:, 0:PACK_CRIT])
        nc.sync.dma_start(out=pack_sb[:, PACK_CRIT:], in_=d_pack[:, PACK_CRIT:])
        o0 = 0
        eh_sb = pack_sb[:, o0:o0 + NH]; o0 += NH
        xq_sb = pack_sb[:, o0:o0 + HW]; o0 += HW
        wt_sb = pack_sb[:, o0:o0 + C]; o0 += C
        qh_sb = pack_sb[:, o0:o0 + NH]; o0 += NH
        eqh_sb = pack_sb[:, o0:o0 + NH]; o0 += NH
        mask_sb = pack_sb[:, o0:o0 + NH]; o0 += NH
        xe_sb = pack_sb[:, o0:o0 + HW]; o0 += HW
        wconv_sb = pack_sb[:, o0:o0 + 9 * C].rearrange(
            "p (t o) -> p t o", t=9); o0 += 9 * C
        gb_sb = consts.tile([C, 2], F32)
        nc.sync.dma_start(out=gb_sb[:], in_=d_gb[:])
        gamma_sb = gb_sb[:, 0:1]
        beta_sb = gb_sb[:, 1:2]

        alpha_sb = consts.tile([C, 1], F32)
        nc.vector.memset(alpha_sb[:], LEAKY)
        eps_sb = consts.tile([C, 1], F32)
        nc.vector.memset(eps_sb[:], BN_EPS)
        # warm the ACT exp table while the input DMAs run
        warm_sb = consts.tile([C, 1], F32)
        nc.scalar.activation(out=warm_sb[:], in_=eps_sb[:],
                             func=mybir.ActivationFunctionType.Exp)

        # ---- EL-half = W_lin @ eh (gates the first attention strip) ----
        elf_sb = big.tile([C, HW], BF16)        # full-m EL for orientation 2 lhsT
        elh_sb = big.tile([C, NH], BF16)        # half-n EL for orientation 1 rhs
        for (off, nb) in N_BLOCKS:
            ps_el = ps_strip.tile([128, 3, 512], F32, tag="sp")
            nc.tensor.matmul(ps_el[0:C, 0, 0:nb], wt_sb[:],
                             eh_sb[:, off:off + nb], start=True, stop=True)
            nc.vector.tensor_copy(elh_sb[:, off:off + nb], ps_el[0:C, 0, 0:nb])

        # [Q^T | 1] via xbar DMA transpose straight from DRAM
        qtd = big.tile([128, M_CHUNKS, C], BF16)
        etd = big.tile([128, M_CHUNKS, C], BF16)
        nc.sync.dma_start_transpose(out=qtd[:], in_=d_xq[:])
        qt_sb = big.tile([128, M_CHUNKS, C + 1], BF16)
        et_sb = big.tile([128, M_CHUNKS, C + 1], BF16)
        nc.vector.memset(qt_sb[:, :, C:C + 1], 1.0)
        nc.vector.tensor_copy(qt_sb[:, :, 0:C], qtd[:])

        def emit_o1_prep():
            # orientation-1 operands; emitted inside the o=0 window of block 0.
            # Their PSUM lives in the (idle during o=0) pv1 slot.
            for j in range(HW // 512):
                ps_el = ps_pv.tile([C + 1, 512], F32, tag="pv1")
                nc.tensor.matmul(ps_el[0:C, :], wt_sb[:],
                                 xe_sb[:, j * 512:(j + 1) * 512],
                                 start=True, stop=True)
                nc.vector.tensor_copy(elf_sb[:, j * 512:(j + 1) * 512],
                                      ps_el[0:C, :])
            nc.sync.dma_start_transpose(out=etd[:], in_=d_xe[:])
            nc.vector.memset(et_sb[:, :, C:C + 1], 1.0)
            nc.vector.tensor_copy(et_sb[:, :, 0:C], etd[:])

        # ---- conv input (built incrementally): [64, 34 rows, 66 cols] ----
        xpad = big.tile([C, 34, 66], BF16)
        nc.vector.memset(xpad[:], 0.0)

        # ---- attention: both orientations, streamed over m in 3-chunk strips ----
        s_lhs = (xq_sb, elf_sb)       # T1[m,l] = sum_c Q[c,m] ELh[c,l] ; T2 = sum_c EL[c,m] qh[c,l]
        s_rhs = (elh_sb, qh_sb)
        pv_lhs = (qt_sb, et_sb)
        # (chunk0, nchunks) strips; chunk width matches the n-block width so
        # one strip tile (3 PSUM banks) always holds nchunks * nb <= 1536 elems
        STRIPS_512 = [(0, 3), (3, 3), (6, 3), (9, 3), (12, 3), (15, 3), (18, 3),
                      (21, 3), (24, 3), (27, 3), (30, 2)]
        STRIPS_128 = [(0, 12), (12, 12), (24, 8)]
        y_sb = big.tile([C, NOUT], F32)
        st = smalls.tile([C, 4, 6], F32, tag="st")

        def emit_conv_block(rb, pool):
            # conv out rows 8rb..8rb+7 <- xpad rows 8rb..8rb+9
            yp = pool.tile([C + 1, 512], F32, tag="yp")
            for tap in range(9):
                dy, dx = tap // 3, tap % 3
                nc.tensor.matmul(
                    yp[0:C, :],
                    wconv_sb[:, tap, :],
                    xpad[:, 8 * rb + dy:8 * rb + dy + 8, dx:dx + 64],
                    start=(tap == 0), stop=(tap == 8))
            nc.vector.tensor_copy(y_sb[:, rb * 512:(rb + 1) * 512], yp[0:C, :])
            nc.vector.bn_stats(out=st[:, rb, :],
                               in_=y_sb[:, rb * 512:(rb + 1) * 512])

        for ib, (off, nb) in enumerate(N_BLOCKS):
            nrows = nb // W_IMG
            r0 = off // W_IMG
            strips = STRIPS_512 if nb == 512 else STRIPS_128
            nsub = 3 if nb == 512 else 12
            pvs = []
            for o in (0, 1):
                pv = ps_pv.tile([C + 1, 512], F32, tag=f"pv{o}")
                pvs.append(pv)
                for (c0, ns) in strips:
                    sp_flat = ps_strip.tile([128, 1536], F32, tag="sp")
                    sp = sp_flat.rearrange("p (a b) -> p a b", b=nb)
                    for u in range(ns):
                        j = c0 + u
                        nc.tensor.matmul(sp[:, u, :],
                                         s_lhs[o][:, 128 * j:128 * j + 128],
                                         s_rhs[o][:, off:off + nb],
                                         start=True, stop=True)
                    ex_flat = expp.tile([128, 1536], BF16, tag="ex")
                    ex = ex_flat.rearrange("p (a b) -> p a b", b=nb)
                    nc.scalar.activation(out=ex[:, 0:ns, :],
                                         in_=sp[:, 0:ns, :],
                                         func=mybir.ActivationFunctionType.Exp)
                    for u in range(ns):
                        j = c0 + u
                        nc.tensor.matmul(pv[:, 0:nb], pv_lhs[o][:, j, :],
                                         ex[:, u, :],
                                         start=(j == 0), stop=(j == M_CHUNKS - 1))
                if off == 0 and o == 0:
                    emit_o1_prep()
            # normalize:  O[c, l] * (1/D[l]) ; 1/D broadcast via gpsimd.
            # Copy PSUM->SBUF first so the accumulator bank frees quickly.
            zs = []
            for o in (0, 1):
                pvc = smalls.tile([C + 1, 512], F32, tag=f"pvc{o}")
                nc.vector.tensor_copy(pvc[:, 0:nb], pvs[o][:, 0:nb])
                rd = smalls.tile([128, 512], F32, tag="rd")
                nc.vector.reciprocal(rd[64:65, 0:nb], pvc[C:C + 1, 0:nb])
                rd0 = smalls.tile([1, 512], F32, tag="rd0")
                nc.sync.dma_start(out=rd0[0:1, 0:nb], in_=rd[64:65, 0:nb])
                bc = smalls.tile([C, 512], F32, tag="bc")
                nc.gpsimd.partition_broadcast(bc[:, 0:nb], rd0[0:1, 0:nb])
                z = smalls.tile([C, 512], F32, tag=f"z{o}")
                nc.vector.tensor_mul(z[:, 0:nb], pvc[0:C, 0:nb], bc[:, 0:nb])
                zs.append(z)
            zsum = smalls.tile([C, 512], F32, tag="zsum")
            nc.vector.tensor_add(zsum[:, 0:nb], zs[0][:, 0:nb], zs[1][:, 0:nb])
            nc.vector.tensor_add(zsum[:, 0:nb], zsum[:, 0:nb],
                                 eqh_sb[:, off:off + nb])
            # masked cast into the padded conv input
            nc.vector.scalar_tensor_tensor(
                out=xpad[:, r0:r0 + nrows, 1:65],
                in0=zsum[:, 0:nb].rearrange("p (r w) -> p r w", w=W_IMG),
                scalar=1.0,
                in1=mask_sb[:, off:off + nb].rearrange("p (r w) -> p r w", w=W_IMG),
                op0=mybir.AluOpType.mult,
                op1=mybir.AluOpType.mult,
            )
        # ---- conv 3x3 + BN stats (after attention; PSUM pools swapped) ----
        ps_pv_cm.__exit__(None, None, None)
        ps_strip_cm.__exit__(None, None, None)
        ps_conv = ctx.enter_context(tc.tile_pool(name="ps_conv", bufs=2, space="PSUM"))
        for rb in range(4):
            emit_conv_block(rb, ps_conv)
        mv = smalls.tile([C, 2], F32, tag="mv")
        nc.vector.bn_aggr(out=mv[:], in_=st[:])

        # ---- BN stats AllGather (cheaper than AllReduce) + local reduce ----
        ccs = smalls.tile([C, 2], F32, tag="ccs")
        m2 = smalls.tile([C, 1], F32, tag="m2")
        nc.vector.tensor_mul(m2[:], mv[:, 0:1], mv[:, 0:1])
        nc.vector.tensor_copy(ccs[:, 0:1], mv[:, 0:1])
        nc.vector.tensor_add(ccs[:, 1:2], mv[:, 1:2], m2[:])
        cc_in = dram.tile([C, 2], F32)
        cc_out = dram.tile([N_CORES, C, 2], F32, addr_space="Shared")
        nc.sync.dma_start(out=cc_in[:], in_=ccs[:])
        nc.gpsimd.collective_compute(
            "AllGather", mybir.AluOpType.bypass,
            replica_groups=[list(range(N_CORES))],
            ins=[cc_in.opt()], outs=[cc_out.opt()])
        gath = smalls.tile([C, 2, N_CORES], F32, tag="gath")
        nc.sync.dma_start(out=gath[:],
                          in_=cc_out[:].rearrange("r c v -> c v r"))
        red = smalls.tile([C, 2], F32, tag="red")
        nc.vector.tensor_reduce(red[:], gath[:], axis=mybir.AxisListType.X,
                                op=mybir.AluOpType.add)

        # mu = red0/8 ; var = red1/8 - mu^2 ; rstd = exp(-0.5*ln(var+eps))
        nc.vector.tensor_scalar_mul(red[:], red[:], 1.0 / N_CORES)
        mu = red[:, 0:1]
        var = smalls.tile([C, 1], F32, tag="var")
        mu2 = smalls.tile([C, 1], F32, tag="mu2")
        nc.vector.tensor_mul(mu2[:], mu, mu)
        nc.vector.tensor_sub(var[:], red[:, 1:2], mu2[:])
        lnv = smalls.tile([C, 1], F32, tag="lnv")
        nc.scalar.activation(out=lnv[:], in_=var[:],
                             func=mybir.ActivationFunctionType.Ln, bias=eps_sb[:])
        rstd = smalls.tile([C, 1], F32, tag="rstd")
        nc.scalar.activation(out=rstd[:], in_=lnv[:],
                             func=mybir.ActivationFunctionType.Exp, scale=-0.5)
        scale_f = smalls.tile([C, 1], F32, tag="scale_f")
        bias_f = smalls.tile([C, 1], F32, tag="bias_f")
        nc.vector.tensor_mul(scale_f[:], gamma_sb[:], rstd[:])
        nc.vector.tensor_mul(bias_f[:], mu, scale_f[:])
        nc.vector.tensor_sub(bias_f[:], beta_sb[:], bias_f[:])

        # ---- apply BN + leaky relu, write out ----
        # Prelu respects a per-partition alpha AP (Lrelu ignores its alpha
        # and uses the hardware default 0.01) -> one ACT op per block.
        osb = big.tile([C, NOUT], F32)
        nc.scalar.activation(out=osb[:], in_=y_sb[:],
                             func=mybir.ActivationFunctionType.Prelu,
                             bias=bias_f[:], scale=scale_f[:],
                             alpha=alpha_sb[:])
        nc.sync.dma_start(out=d_out[:], in_=osb[:])

    nc.compile()
    return nc


def _get_program():
    global _COMPILED
    if _COMPILED is None:
        _COMPILED = _build_program()
    return _COMPILED


def _make_in_maps(exemplar, query, W_lin, W_conv, gamma, beta):
    E = np.asarray(exemplar, dtype=np.float32).reshape(4, C, HW)
    Q = np.asarray(query, dtype=np.float32).reshape(4, C, HW)
    wt = np.ascontiguousarray(np.asarray(W_lin, np.float32).T).astype(NPBF16)
    assert wt.shape == (C, C)
    wconv = np.ascontiguousarray(
        np.asarray(W_conv, np.float32).transpose(1, 2, 3, 0).reshape(C, 9, C)
    ).astype(NPBF16)
    g = np.asarray(gamma, np.float32).reshape(C, 1)
    b = np.asarray(beta, np.float32).reshape(C, 1)

    zeros = np.zeros((C, W_IMG), np.float32)
    in_maps = []
    for k in range(N_CORES):
        s, h = divmod(k, 2)
        if h == 0:
            sl = lambda X: np.concatenate([zeros, X[s][:, :NH - W_IMG]], axis=1)
        else:
            sl = lambda X: np.concatenate([X[s][:, HW - (NH - W_IMG):], zeros], axis=1)
        eh = sl(E)
        qh = sl(Q)
        mask = np.ones((C, NH), np.float32)
        if h == 0:
            mask[:, :W_IMG] = 0.0
        else:
            mask[:, NH - W_IMG:] = 0.0
        xe_bf = E[s].astype(NPBF16)
        xq_bf = Q[s].astype(NPBF16)
        # order must match the device-side unpack:
        #   [eh | xq | wt | qh | eqh | mask | xe | wconv]
        pack = np.concatenate([
            eh.astype(NPBF16), xq_bf, wt, qh.astype(NPBF16),
            (eh + qh).astype(NPBF16), mask.astype(NPBF16),
            xe_bf, wconv.reshape(C, 9 * C),
        ], axis=1)
        in_maps.append({
            "pack": np.ascontiguousarray(pack),
            "xe": xe_bf,
            "xq": xq_bf,
            "gb": np.ascontiguousarray(np.concatenate([g, b], axis=1)),
        })
    return in_maps


def kernel(exemplar, query, W_lin, W_conv, gamma, beta):
    nc = _get_program()
    in_maps = _make_in_maps(exemplar, query, W_lin, W_conv, gamma, beta)
    res = bass_utils.run_bass_kernel_spmd(
        nc, in_maps, core_ids=list(range(N_CORES)), trace=False)
    out = np.empty((4, C, 64, 64), np.float32)
    for k in range(N_CORES):
        s, h = divmod(k, 2)
        out[s, :, 32 * h:32 * h + 32, :] = \
            res.results[k]["out"].reshape(C, 32, 64)
    return out
